# revision 77
# baseline (speedup 1.0000x reference)
"""Trainium2 Bass kernel for EnhancedConditionalUNet forward (B=64, 8 cores data-parallel).

Self-contained: hardcodes all shapes. kernel(**inputs) -> np.ndarray [64,3,64,64] f32.

v2 design (vs fp16 baseline, ~548us -> ~480us):
- attention block (q/k/v projections, scores, exp, colsum+replicate, attn-out)
  in fp8e4m3; DoubleRow packs 2 K-tiles per matmul (2x K per instruction at the
  same 1 col/cycle rate = 2x FLOPs); power-of-2 scale management (acts x2^5,
  attn weights x2^6) keeps evictions 2-op; colsum is fused with its row
  replication via a [128,2,128] ones lhsT DR matmul feeding one reciprocal
- conv backbone stays fp16 (fp8 fails the 2e-2 gate: each conv activation or
  weight site alone costs ~2.3e-2), but e2 and dc get physical tap-pairing: a
  shifted duplicate of the input activation on partitions 64-127 (SBUF->SBUF
  DMA) doubles K for the K<=64 layers
- dc interleaves its 4 tile_position column groups tap-major: narrow-M (M=3)
  matmuls in different <=32-wide column tiles overlap ~3x on the PE
- emission order interleaves sample s's serial attention chain with sample
  s-1's back-half convs so the PE never drains (p-state stays at 2.4GHz)

v3 (DMA/overlap round; timed via repeat-delta, profiled on CoreSim timeline):
- DMA cost is fixed-overhead dominated (~625ns global HWDGE hold + ~650-900ns
  latency per transfer), so: m0 loads merged 4->1 DMA per sample alternating
  SP/ACT queues; h1d dups merged 4->2; h5d dup merged 4->1 by padding rows to
  67 so one flat contiguous-run copy covers all rows (64 descriptors instead
  of 2048 strided ones); per-sample output DMAs moved to the gpsimd SWDGE
- all biases/scales packed into one [128,16] f32 table (one DMA, was 10)
- bordered activation tiles are persistent double-buffers (index s%2) and
  only their border cells are zeroed: the full-tile memsets serialized DVE
  for ~21us at startup, stalling e2's eviction and re-throttling the PE
- prev-sample dc matmul units interleaved between the scores/exp chunks of
  the current sample (engine FIFOs stall on PSUM-bank waits otherwise); dc
  tail restructured so output DMAs overlap the remaining tanh calls

v4 (sim span 443.6us -> 395.9us; rel err 1.32e-3):
- dc accumulation reordered singles-first: the wdcs single-tap matmuls read
  only partitions 0:64, so they run while the h5d dup (which every wdcp pair
  matmul needs) is still in flight — removed the dup from the critical path
  in both the steady state and the epilogue
- early scores chunks are filled with the NEXT sample's first four e1 rows
  (m0 is prefetched; evictions on DVE which is idle there), since dc units
  can't start until the dup lands; remaining dc units interleave from cc2
- softmax reciprocal via single-op reciprocal_approx_fast (18-bit accuracy,
  fine for denominators; halves the DVE recip cost and the at->b2 chain)
- last two samples' output DMAs on the HWDGE queues (the single SWDGE ring
  serialized 4x1.6us after the final matmul); s0 h1d dups split per-r on the
  idle Pool queue so e2(s0) starts ~2.5us earlier
- NOTE: fp8 for the conv backbone is a dead end at this 2e-2 gate — fp8
  dot-product noise is ~3-6% of the OUTPUT independent of K (noise terms
  scale with the same terms that form the sum), only the gamma=0.1-suppressed
  attention path tolerates it. Engine moves that put work on DVE during the
  rep->recip->t1->hb2 window, or evictions on Pool (0.42 eff), regressed.

v5 (sim span 395.9us -> 388.0us; DVE-backlog round):
- qkv moved back before dt[2,3]: the v-projection PSUM evictions (DVE) now
  overlap dt's PE work instead of stalling the v matmuls on psS recycling
- early-emitted e1 rows evict on DVE (ACT is saturated by exp there; gpsimd
  is ILLEGAL for this — GPSIMD cannot access PSUM, the BIR verifier rejects
  the NEFF even though CoreSim happily simulates it)
- e2 r=0 emitted right after rep_at (its inputs exist from the early e1
  rows + first dup) with ACT evictions: covers b2's 1us wait on the
  recip->t1->hb2 DVE chain without lengthening that chain
- last-two-samples' output DMAs split sync/Pool (never the ACT queue: the
  DMA seq slot would sit between the final tanh calls and delay them)

v6 (sim span 389.1us -> 382.7us):
- v-projection evictions on ACT via AF.Copy (identity-with-scale; ACT may
  read PSUM, gpsimd may not): the DVE backlog behind the fast DR matmul
  pairs was stalling psS recycling. All-ACT beat an ACT/DVE alternation.
- s0's S phase (no prev-dc units) now absorbs all 8 of s1's e1 rows
- tried and reverted: splitting s0's m0 load / moving s0 dups to sync
  (queue congestion + longer HAM cold phase), alternating v evictions.
v7: PE warm-up — 22 dummy fp8 matmuls on the ones8 constant fill the
otherwise-idle window before the first m0 DMA lands, so the HAM clock gate
reaches 2.4GHz before the real e1 matmuls start (sim-span-neutral: the
startup critical path is DMA latency; on HW it removes the cold-clock
first-sample penalty). S-phase row-tiling was costed and rejected: the k/q
replication DMAs (+5us/sample of queue time) exceed the ~2.5us of HW
column-overlap they would buy.

Remaining known floor: ~6.8us final drain (4 serial tanh + out DMA latency;
only recoverable by Q-major dc ordering for the last sample, which would
forfeit the HW column-tile overlap the sim cannot see), ~7us startup DMA
arrival, one-time s0/s7 phase edges, and the structural PE work itself.
"""
import numpy as np
import ml_dtypes

import concourse.bass as bass
import concourse.tile as tile
from concourse import bacc, mybir
from concourse.bass_utils import run_bass_kernel_spmd

NCORES = 8
NS = 8          # samples per core
BF = mybir.dt.bfloat16
F16 = mybir.dt.float16
F32 = mybir.dt.float32
F8 = mybir.dt.float8e4
NPF8 = ml_dtypes.float8_e4m3
AF = mybir.ActivationFunctionType
OP = mybir.AluOpType
DR = mybir.MatmulPerfMode.DoubleRow

_cache = {}


def build_nc(ns=NS, repeat=1):
    nc = bacc.Bacc("TRN2", target_bir_lowering=False, debug=False)

    d_m0 = nc.dram_tensor("m0", [ns, 36, 64, 64], F16, kind="ExternalInput")
    d_wim = nc.dram_tensor("wim", [36, 64], F16, kind="ExternalInput")
    d_wep = nc.dram_tensor("wep", [128, 3, 128], F16, kind="ExternalInput")
    d_wes = nc.dram_tensor("wes", [64, 3, 128], F16, kind="ExternalInput")
    d_wb1 = nc.dram_tensor("wb1", [128, 9, 2, 128], F16, kind="ExternalInput")
    d_wqk8 = nc.dram_tensor("wqk8", [128, 2, 64], F8, kind="ExternalInput")
    d_wv8 = nc.dram_tensor("wv8", [128, 2, 256], F8, kind="ExternalInput")
    d_wb2 = nc.dram_tensor("wb2", [128, 2, 9, 128], F16, kind="ExternalInput")
    d_wdt = nc.dram_tensor("wdt", [128, 16, 64], F16, kind="ExternalInput")
    d_wdc = nc.dram_tensor("wdc", [128, 18], F16, kind="ExternalInput")
    d_bias = nc.dram_tensor("biases", [128, 16], F32, kind="ExternalInput")
    d_out = nc.dram_tensor("out", [ns, 3, 64, 64], F32, kind="ExternalOutput")

    with tile.TileContext(nc) as tc:
        with (
            tc.tile_pool(name="wpool", bufs=1) as wp,
            tc.tile_pool(name="apool", bufs=2) as ap,
            tc.tile_pool(name="spool", bufs=1) as sp,
            tc.tile_pool(name="psS", bufs=4, space="PSUM") as psS,
            tc.tile_pool(name="psB", bufs=2, space="PSUM") as psB,
        ):
            def wload(name, shape, dt, dram, eng):
                t = wp.tile(shape, dt, name=name)
                eng.dma_start(t[:], dram[:])
                return t

            _m0pre = {}

            def m0_load(s):
                m0 = ap.tile([36, 64, 64], F16, name="m0t", bufs=3)
                eng = nc.sync if s % 2 == 0 else nc.scalar
                eng.dma_start(m0[:, :, :], d_m0[s])
                return m0

            wim = wload("wim", [36, 64], F16, d_wim, nc.scalar)
            biasT = wload("biasT", [128, 16], F32, d_bias, nc.scalar)
            _m0pre[0] = m0_load(0)
            wep = wload("wep", [128, 3, 128], F16, d_wep, nc.scalar)
            wes = wload("wes", [64, 3, 128], F16, d_wes, nc.scalar)
            wb1 = wload("wb1", [128, 9, 2, 128], F16, d_wb1, nc.sync)
            wqk8 = wload("wqk8", [128, 2, 64], F8, d_wqk8, nc.gpsimd)
            wv8 = wload("wv8", [128, 2, 256], F8, d_wv8, nc.gpsimd)
            ones8 = wp.tile([128, 2, 128], F8)
            nc.vector.memset(ones8[:], 1.0)

            # PE warm-up: the HAM clock gate holds the PE at half clock until
            # ~3.4us of sustained activity. The PE would idle until the first
            # m0 DMA lands (~6us) anyway, so burn dummy matmuls on the ones8
            # constant; the real e1/e2 matmuls then start at full clock.
            warm_ps = psS.tile([128, 512], F32, name="pcs")
            ones_r = ones8.rearrange("p a b -> p (a b)")
            for _ in range(22):
                nc.tensor.matmul(warm_ps[:, 0:256], ones_r[:, 0:128],
                                 ones_r[:], start=True, stop=True)

            E = ap.tile([128, 8, 1024], F8, name="E", bufs=1)
            invg = sp.tile([128, 1024], F32, name="invg")
            scr = sp.tile([128, 1024], F32, name="scr")
            t1 = sp.tile([128, 1024], F32, name="t1")

            def zero_all(t):
                nc.vector.memset(t[:], 0.0)

            taps = [(dy, dx) for dy in range(3) for dx in range(3)]

            def f_e1(c, rs, evict_dve=False):
                """e1: im2col matmuls -> relu -> h1d[:64]; dup [64:] via DMA."""
                s, h1d = c["s"], c["h1d"]
                if rs[0] == 0:
                    m0 = _m0pre.pop(s, None)
                    if m0 is None:
                        m0 = m0_load(s)
                    c["m0"] = m0
                m0 = c["m0"]
                h1f = h1d.rearrange("p a b c -> p a (b c)")
                h1flat = h1d.rearrange("p a b c -> p (a b c)")
                for r in rs:
                    ps = psS.tile([128, 512], F32, name="pcs")
                    nc.tensor.matmul(ps[0:64], wim[:], m0[:, 8 * r:8 * r + 8, :],
                                     start=True, stop=True)
                    if evict_dve or (s == 0 and r % 2 == 1):
                        # DVE eviction where ACT is contended (exp phase /
                        # s=0 startup). NOTE: gpsimd is NOT an option here —
                        # GPSIMD instructions cannot access PSUM (BIR
                        # verifier rejects it; CoreSim doesn't check).
                        nc.vector.tensor_scalar(
                            out=h1f[0:64, 1 + 8 * r:9 + 8 * r, 1:65],
                            in0=ps[0:64].rearrange("p (a b) -> p a b", a=8),
                            scalar1=biasT[0:64, 0:1], scalar2=0.0,
                            op0=OP.add, op1=OP.max)
                    else:
                        nc.scalar.activation(
                            h1f[0:64, 1 + 8 * r:9 + 8 * r, 1:65],
                            ps[0:64].rearrange("p (a b) -> p a b", a=8),
                            AF.Relu, bias=biasT[0:64, 0:1], scale=1.0)
                    if s == 0 and r % 2 == 1:
                        # startup: finer-grained dups so e2 can begin as soon
                        # as rows 1..32 exist (trim 33 tail elems that would
                        # read the next not-yet-written row; the dropped dst
                        # cells are unread plane-1 slots)
                        o0 = (8 * r - 7) * 66
                        nn = 1056 if r == 7 else 1023
                        nc.gpsimd.dma_start(
                            h1flat[64:128, o0:o0 + nn],
                            h1flat[0:64, o0 + 33:o0 + 33 + nn])
                    elif s > 0 and r in (3, 7):
                        # steady state: two merged dups (fewer DMAs win)
                        o0, nn = (66, 2079) if r == 3 else (33 * 66, 2112)
                        nc.gpsimd.dma_start(
                            h1flat[64:128, o0:o0 + nn],
                            h1flat[0:64, o0 + 33:o0 + 33 + nn])

            def f_e2(c, rr):
                h1d, h2 = c["h1d"], c["h2"]
                for r in rr:
                    ps = psS.tile([128, 512], F32, name="pcs")
                    n = 0
                    for dy in range(3):
                        nc.tensor.matmul(ps[:], wep[:, dy, :],
                                         h1d[:, dy + 32 * r:dy + 32 * r + 32:2, 0, 0:32],
                                         start=(n == 0), stop=False)
                        n += 1
                    for dy in range(3):
                        n += 1
                        nc.tensor.matmul(ps[:], wes[:, dy, :],
                                         h1d[0:64, dy + 32 * r:dy + 32 * r + 32:2, 0, 1:33],
                                         start=False, stop=(n == 6))
                    # ACT eviction: e2 r=0 is emitted inside the attention
                    # window to cover b2's wait on the DVE chain — a DVE
                    # eviction here would lengthen that very chain
                    nc.scalar.activation(h2[:, 1 + 16 * r:17 + 16 * r, 1:33],
                                         ps[:], AF.Relu, bias=biasT[:, 1:2],
                                         scale=1.0)

            def f_b1(c):
                h2 = c["h2"]
                h3f16, h3f8 = c["h3f16"], c["h3f8"]
                for mh in range(2):
                    for r in range(2):
                        ps = psS.tile([128, 512], F32, name="pcs")
                        for ti, (dy, dx) in enumerate(taps):
                            nc.tensor.matmul(
                                ps[:], wb1[:, ti, mh, :],
                                h2[:, dy + 16 * r:dy + 16 * r + 16, dx:dx + 32],
                                start=(ti == 0), stop=(ti == 8))
                        nc.scalar.activation(
                            h3f8[:, mh, 512 * r:512 * r + 512], ps[:],
                            AF.Relu, bias=biasT[:, 4 + mh:5 + mh], scale=32.0)
                        nc.vector.tensor_scalar(
                            out=h3f16[:, mh, 512 * r:512 * r + 512],
                            in0=ps[:], scalar1=biasT[:, 2 + mh:3 + mh],
                            scalar2=0.0, op0=OP.add, op1=OP.max)

            def f_qkv(c):
                """qk fused DR matmul + k relayout DMA; v DR matmuls -> vT8."""
                h3f8, qk8, ksb, vT8 = c["h3f8"], c["qk8"], c["ksb"], c["vT8"]
                for r in range(2):
                    ps = psS.tile([128, 512], F32, name="pcs")
                    nc.tensor.matmul(ps[0:64], wqk8[:], h3f8[:, :, 512 * r:512 * r + 512],
                                     start=True, stop=True, perf_mode=DR)
                    nc.vector.tensor_scalar(out=qk8[:, 512 * r:512 * r + 512],
                                            in0=ps[0:64], scalar1=biasT[0:64, 6:7],
                                            scalar2=2.0 ** -5, op0=OP.add, op1=OP.mult)
                    nc.gpsimd.dma_start(ksb[:, 512 * r:512 * r + 512],
                                        qk8[32:64, 512 * r:512 * r + 512])
                for cc2 in range(4):
                    ps = psS.tile([128, 512], F32, name="pcs")
                    for u in range(2):
                        cc = 2 * cc2 + u
                        nc.tensor.matmul(ps[:, 256 * u:256 * u + 256],
                                         h3f8[:, :, 128 * cc:128 * cc + 128],
                                         wv8[:], start=True, stop=True, perf_mode=DR)
                    # ACT Copy (identity with scale): keeps the v evictions
                    # off DVE, whose backlog stalls psS recycling here
                    nc.scalar.activation(vT8[:, 2 * cc2:2 * cc2 + 2, :],
                                         ps[:], AF.Copy, scale=2.0 ** -6)

            def f_S(c, pc=None, defer=0, nxt=None):
                """scores (fp8, K=32) + exp -> E fp8. Fillers between chunks:
                cc1-2 emit the NEXT sample's first e1 rows (the h5d dup that
                gates every dc unit is still in flight then); cc3+ interleave
                prev-sample dc units. Returns `defer` unemitted units."""
                qk8, ksb, E = c["qk8"], c["ksb"], c["E"]
                units = dc_units(pc) if pc is not None else []
                stop = len(units) - defer
                ui = 0
                for cc in range(8):
                    sps = psB.tile([128, 1024], F32, name="pbig")
                    for ih in range(2):
                        nc.tensor.matmul(sps[:, 512 * ih:512 * ih + 512],
                                         ksb[:, 128 * cc:128 * cc + 128],
                                         qk8[0:32, 512 * ih:512 * ih + 512],
                                         start=True, stop=True)
                    nc.scalar.activation(E[:, cc, :], sps[:], AF.Exp, scale=2.0 ** -12)
                    if nxt is not None and (cc in (1, 2) or
                                            (pc is None and cc in (3, 4))):
                        # s0's S phase has no prev-dc units: emit all 8 of
                        # the next sample's e1 rows there instead of 4
                        f_e1(nxt, [2 * cc - 2, 2 * cc - 1], evict_dve=True)
                    take = {0: 0, 1: 0, 2: 1, 7: 1}.get(cc, 2)
                    for _ in range(take):
                        if ui < stop:
                            units[ui]()
                            ui += 1
                while ui < stop:
                    units[ui]()
                    ui += 1
                return units[stop:]

            def f_rep_at(c):
                """colsum-rep, reciprocal, attn out, normalize -> hb2."""
                E, vT8, h3f16, hb2 = c["E"], c["vT8"], c["h3f16"], c["hb2"]
                rep = psB.tile([128, 1024], F32, name="pbig")
                for ih in range(2):
                    for cp in range(4):
                        nc.tensor.matmul(rep[:, 512 * ih:512 * ih + 512], ones8[:],
                                         E[:, 2 * cp:2 * cp + 2, 512 * ih:512 * ih + 512],
                                         start=(cp == 0), stop=(cp == 3), perf_mode=DR)
                nc.vector.reciprocal_approx_fast(invg[:], rep[:])
                for ch in range(2):
                    at = psB.tile([128, 1024], F32, name="pbig")
                    for ih in range(2):
                        for cp in range(4):
                            nc.tensor.matmul(
                                at[:, 512 * ih:512 * ih + 512],
                                vT8[:, 2 * cp:2 * cp + 2, 128 * ch:128 * ch + 128],
                                E[:, 2 * cp:2 * cp + 2, 512 * ih:512 * ih + 512],
                                start=(cp == 0), stop=(cp == 3), perf_mode=DR)
                    nc.vector.scalar_tensor_tensor(
                        out=t1[:], in0=at[:], scalar=biasT[:, 9:10], in1=invg[:],
                        op0=OP.mult, op1=OP.mult)
                    nc.vector.scalar_tensor_tensor(
                        out=hb2[:, ch, 1:33, 1:33],
                        in0=t1[:], scalar=biasT[:, 7 + ch:8 + ch],
                        in1=h3f16[:, ch, :], op0=OP.add, op1=OP.add)

            def f_b2(c):
                hb2, h4 = c["hb2"], c["h4"]
                wb2 = late["wb2"]
                for r in range(2):
                    ps = psS.tile([128, 512], F32, name="pcs")
                    for kh in range(2):
                        for ti, (dy, dx) in enumerate(taps):
                            nc.tensor.matmul(
                                ps[:], wb2[:, kh, ti, :],
                                hb2[:, kh, dy + 16 * r:dy + 16 * r + 16, dx:dx + 32],
                                start=(kh == 0 and ti == 0), stop=(kh == 1 and ti == 8))
                    nc.vector.tensor_scalar(out=h4[:, 1 + 16 * r:17 + 16 * r, 1:33],
                                            in0=ps[:], scalar1=biasT[:, 10:11],
                                            scalar2=0.0, op0=OP.add, op1=OP.max)

            def f_dt(c, phases):
                h4, h5d = c["h4"], c["h5d"]
                wdt = late["wdt"]
                aoff = {(0, 0): 1, (0, 1): 0, (1, 0): 2, (1, 1): 1}
                for ph in phases:
                    py, px = ph // 2, ph % 2
                    if True:
                        for r in range(2):
                            ps = psS.tile([128, 512], F32, name="pcs")
                            ti = 0
                            for dy2 in range(2):
                                for dx2 in range(2):
                                    ay = aoff[(py, dy2)]
                                    ax = aoff[(px, dx2)]
                                    nc.tensor.matmul(
                                        ps[0:64], wdt[:, ph * 4 + dy2 * 2 + dx2, :],
                                        h4[:, ay + 16 * r:ay + 16 * r + 16, ax:ax + 32],
                                        start=(ti == 0), stop=(ti == 3))
                                    ti += 1
                            y0 = 1 + py + 32 * r
                            out_ap = h5d[0:64, y0:y0 + 32:2, 1 + px:1 + px + 64:2]
                            if py == 0:
                                nc.vector.tensor_scalar(out=out_ap, in0=ps[0:64],
                                                        scalar1=biasT[0:64, 11:12],
                                                        scalar2=0.0,
                                                        op0=OP.add, op1=OP.max)
                            else:
                                nc.scalar.activation(out_ap, ps[0:64], AF.Relu,
                                                     bias=biasT[0:64, 11:12],
                                                     scale=1.0)
                    if ph == 3:
                        # all 4 phases written: one contiguous flat-range dup
                        # (row width padded to 67 so runs span row boundaries)
                        deng = nc.gpsimd if c["s"] == ns - 1 else nc.sync
                        h5f = h5d.rearrange("p a b -> p (a b)")
                        deng.dma_start(h5f[64:128, 67:67 + 4288],
                                       h5f[0:64, 68:68 + 4288])

            def dc_units(c):
                """dc as a list of emit-closures so f_S can interleave them."""
                s, h5d, dct = c["s"], c["h5d"], c["dct"]
                wdc = late["wdc"]
                units = []
                qts = {}

                def mk_mm(r2, n):
                    # singles (n=3..5, read only partitions 0:64) are emitted
                    # BEFORE the pairs (n=0..2, need the h5d dup partitions):
                    # the dup's latency hides under the single-tap matmuls
                    def emit():
                        if n == 3:
                            qts[r2] = psS.tile([128, 512], F32, name="pcs")
                        qt = qts[r2]
                        dy = n if n < 3 else n - 3
                        # tap-major order: consecutive matmuls hit different
                        # PE column tiles and overlap ~3x
                        for Q in range(4):
                            y0 = 16 * Q + 8 * r2
                            if n < 3:
                                nc.tensor.matmul(
                                    qt[32 * Q:32 * Q + 3, :],
                                    wdc[:, 3 * dy:3 * dy + 3],
                                    h5d[:, dy + y0:dy + y0 + 8, 0:64],
                                    start=False, stop=(n == 2),
                                    tile_position=(0, 32 * Q))
                            else:
                                nc.tensor.matmul(
                                    qt[32 * Q:32 * Q + 3, :],
                                    wdc[0:64, 9 + 3 * dy:12 + 3 * dy],
                                    h5d[0:64, dy + y0:dy + y0 + 8, 2:66],
                                    start=(n == 3), stop=False,
                                    tile_position=(0, 32 * Q))
                    return emit

                def mk_tanh(r2):
                    def emit():
                        qt = qts[r2]
                        for Q in range(4):
                            nc.scalar.activation(
                                dct[32 * Q:32 * Q + 3, 512 * r2:512 * r2 + 512],
                                qt[32 * Q:32 * Q + 3, :], AF.Tanh,
                                bias=biasT[32 * Q:32 * Q + 3, 12:13], scale=1.0)
                    return emit

                def mk_tanhout(Q):
                    # r2=1 tanh for this Q, then its output DMA — so the out
                    # DMAs overlap the remaining tanh calls at the tail. The
                    # last two samples' outputs go on the HWDGE queues (idle
                    # at the end; the single SWDGE ring would serialize them
                    # after the final matmul).
                    def emit():
                        qt = qts[1]
                        nc.scalar.activation(
                            dct[32 * Q:32 * Q + 3, 512:1024],
                            qt[32 * Q:32 * Q + 3, :], AF.Tanh,
                            bias=biasT[32 * Q:32 * Q + 3, 12:13], scale=1.0)
                        # sync/Pool mix (not scalar: a DMA on the ACT queue
                        # would sit between the final tanh calls): two idle
                        # queues drain the last outputs in parallel
                        if s >= ns - 2:
                            deng = nc.sync if Q % 2 == 0 else nc.gpsimd
                        else:
                            deng = nc.gpsimd
                        deng.dma_start(
                            d_out[s][:, 16 * Q:16 * Q + 16, :],
                            dct[32 * Q:32 * Q + 3, :])
                    return emit

                for n in (3, 4, 5, 0, 1, 2):
                    units.append(mk_mm(0, n))
                units.append(mk_tanh(0))
                for n in (3, 4, 5, 0, 1, 2):
                    units.append(mk_mm(1, n))
                for Q in range(4):
                    units.append(mk_tanhout(Q))
                return units

            def f_dc(c):
                for u in dc_units(c):
                    u()

            # bordered tiles: persistent double-buffers (borders stay zero
            # across generations; single tensor id keeps the sim shadow happy)
            perst = {}
            for nm, shape in (("h1d", [128, 66, 2, 33]), ("h2", [128, 34, 34]),
                              ("hb2", [128, 2, 34, 34]), ("h4", [128, 34, 34]),
                              ("h5d", [128, 66, 67])):
                perst[nm] = [sp.tile(shape, F16, name=f"{nm}_{i}")
                             for i in range(2)]

            def alloc_ctx(s):
                return dict(
                    s=s,
                    h1d=perst["h1d"][s % 2],
                    h2=perst["h2"][s % 2],
                    h3f16=ap.tile([128, 2, 1024], F16, name="h3f16"),
                    h3f8=ap.tile([128, 2, 1024], F8, name="h3f8"),
                    qk8=ap.tile([64, 1024], F8, name="qk8"),
                    ksb=ap.tile([32, 1024], F8, name="ksb"),
                    vT8=ap.tile([128, 8, 256], F8, name="vT8"),
                    hb2=perst["hb2"][s % 2],
                    h4=perst["h4"][s % 2],
                    h5d=perst["h5d"][s % 2],
                    dct=ap.tile([128, 1024], F32, name="dct"),
                    E=E,
                )

            late = {}
            prev = None
            # emission order interleaves sample s's serial attention chain with
            # sample s-1's back-half conv matmuls so the PE never drains
            # zero only the border cells the convs read (rows/cols outside the
            # written interior) — full-tile memsets serialized DVE for ~21us
            for i in range(2):
                h1d_, h2_, hb2_, h4_, h5d_ = (perst[n][i] for n in
                                              ("h1d", "h2", "hb2", "h4", "h5d"))
                V, G = nc.vector, nc.gpsimd
                V.memset(h1d_[:, 0], 0.0)
                V.memset(h1d_[:, 65], 0.0)
                V.memset(h1d_[:, :, 0, 0:1], 0.0)
                V.memset(h1d_[:, :, 1, 32:33], 0.0)
                for t in (h2_, h4_):
                    G.memset(t[:, 0], 0.0)
                    G.memset(t[:, 33], 0.0)
                    G.memset(t[:, :, 0:1], 0.0)
                    G.memset(t[:, :, 33:34], 0.0)
                G.memset(hb2_[:, :, 0], 0.0)
                G.memset(hb2_[:, :, 33], 0.0)
                G.memset(hb2_[:, :, :, 0:1], 0.0)
                G.memset(hb2_[:, :, :, 33:34], 0.0)
                V.memset(h5d_[:, 0], 0.0)
                V.memset(h5d_[:, 65], 0.0)
                V.memset(h5d_[:, :, 0:1], 0.0)
                V.memset(h5d_[:, :, 65:67], 0.0)
            nxt_box = {}
            for rep in range(repeat):
              for s in range(ns):
                c = nxt_box.pop("c", None)
                pre_rs = nxt_box.pop("pre", 0) if c is not None else 0
                if c is None:
                    c = alloc_ctx(s)
                if prev is not None:
                    f_rep_at(prev)
                if pre_rs >= 4:
                    # rows 0-3 + the first dup were emitted inside the prev
                    # sample's S phase, so e2 r=0 is ready now: it covers
                    # b2's wait on the attention DVE chain (recip/t1/hb2)
                    f_e2(c, [0])
                    if pre_rs < 8:
                        f_e1(c, [4, 5, 6, 7])
                else:
                    f_e1(c, [0, 1, 2, 3, 4, 5, 6, 7])
                if rep == 0 and s == 0:
                    late["wb2"] = wload("wb2", [128, 2, 9, 128], F16, d_wb2,
                                        nc.sync)
                    late["wdt"] = wload("wdt", [128, 16, 64], F16, d_wdt,
                                        nc.scalar)
                    late["wdc"] = wload("wdc", [128, 18], F16, d_wdc, nc.gpsimd)
                if prev is not None:
                    f_b2(prev)
                if pre_rs == 4:
                    f_e2(c, [1])
                else:
                    f_e2(c, [0, 1])
                f_b1(c)
                if prev is not None:
                    f_dt(prev, [0, 1])
                f_qkv(c)
                if prev is not None:
                    f_dt(prev, [2, 3])
                last = (rep == repeat - 1 and s == ns - 1)
                nxt = None
                if s + 1 < ns:
                    nxt = alloc_ctx(s + 1)
                    nxt_box["c"] = nxt
                    nxt_box["pre"] = 8 if prev is None else 4
                leftover = f_S(c, prev, defer=8 if last else 0, nxt=nxt)
                prev = c
            if prev is not None:
                f_rep_at(prev)
                f_b2(prev)
                f_dt(prev, [0, 1, 2, 3])
                for u in leftover:
                    u()
                f_dc(prev)

    nc.compile()
    return nc


def prep_static(ew1, eb1, ew2, eb2, bw1, bb1, qw, qb, kw, kb, vw, vb,
                gamma, bw2, bb2, dtw, dtb, dcw, dcb):
    """Host-side weight layout prep (shared across cores)."""
    f16 = np.float16
    f32 = np.float32
    out = {}
    wim = np.zeros((36, 64), np.float32)
    for dy in range(3):
        for dx in range(3):
            t = dy * 3 + dx
            wim[t * 4:t * 4 + 4, :] = ew1[:, :, dy, dx].T
    out["wim"] = wim.astype(f16)
    # packed per-partition bias/scale table [128, 16] f32:
    # col 0: be1(p0:64) | 1: be2 | 2-3: bb1 | 4-5: bb1s | 6: bqk(p0:64)
    # 7-8: gvb | 9: gam5 | 10: bb2 | 11: bdt(p0:64) | 12: bdc
    biases = np.zeros((128, 16), f32)
    biases[0:64, 0] = eb1
    # e2: paired taps (dy,0)+(dy,1) stacked on K, singles (dy,2)
    we2 = np.transpose(ew2, (1, 2, 3, 0)).reshape(64, 9, 128)  # [cin, tap, cout]
    wep = np.zeros((128, 3, 128), np.float32)
    wes = np.zeros((64, 3, 128), np.float32)
    for dy in range(3):
        wep[0:64, dy, :] = we2[:, dy * 3 + 0, :]
        wep[64:128, dy, :] = we2[:, dy * 3 + 1, :]
        wes[:, dy, :] = we2[:, dy * 3 + 2, :]
    out["wep"] = wep.astype(f16)
    out["wes"] = wes.astype(f16)
    biases[:, 1] = eb2
    wb1 = np.transpose(bw1, (1, 2, 3, 0)).reshape(128, 9, 2, 128)
    out["wb1"] = np.ascontiguousarray(wb1).astype(f16)
    biases[:, 2:4] = bb1.reshape(2, 128).T
    biases[:, 4:6] = biases[:, 2:4] * 32.0
    # attention fp8: wqk8 [128(kp),2(kh),64(m)] = [q cols 0:32 | k cols 32:64] x2^6
    wq_t = qw[:, :, 0, 0].T.reshape(2, 128, 32)   # [kh, kp, c]
    wk_t = kw[:, :, 0, 0].T.reshape(2, 128, 32)
    wqk8 = np.zeros((128, 2, 64), np.float32)
    wqk8[:, :, 0:32] = wq_t.transpose(1, 0, 2) * 64.0
    wqk8[:, :, 32:64] = wk_t.transpose(1, 0, 2) * 64.0
    out["wqk8"] = wqk8.astype(NPF8)
    biases[0:64, 6] = np.concatenate([qb, kb]) * 2.0 ** 11
    wv = vw[:, :, 0, 0].T.reshape(2, 128, 256).transpose(1, 0, 2)
    out["wv8"] = np.ascontiguousarray(wv * 64.0).astype(NPF8)
    g = float(np.asarray(gamma).reshape(-1)[0])
    biases[:, 7:9] = (g * vb).reshape(2, 128).T
    biases[:, 9] = g * 2.0 ** -5
    wb2_ = np.transpose(bw2, (1, 2, 3, 0)).reshape(2, 128, 9, 128).transpose(1, 0, 2, 3)
    out["wb2"] = np.ascontiguousarray(wb2_).astype(f16)
    biases[:, 10] = bb2
    kmap = {(0, 0): 1, (0, 1): 3, (1, 0): 0, (1, 1): 2}
    wdt = np.zeros((128, 16, 64), np.float32)
    for py in range(2):
        for px in range(2):
            for dy2 in range(2):
                for dx2 in range(2):
                    ky = kmap[(py, dy2)]
                    kx = kmap[(px, dx2)]
                    wdt[:, (py * 2 + px) * 4 + dy2 * 2 + dx2, :] = dtw[:, :, ky, kx]
    out["wdt"] = wdt.astype(f16)
    biases[0:64, 11] = dtb
    # dc: paired taps (dy,0)+(dy,1) on K, singles (dy,2)
    wdc = np.transpose(dcw, (1, 2, 3, 0)).reshape(64, 9, 3)  # [cin, tap, cout]
    wdcp = np.zeros((128, 3, 3), np.float32)
    wdcs = np.zeros((64, 3, 3), np.float32)
    for dy in range(3):
        wdcp[0:64, dy, :] = wdc[:, dy * 3 + 0, :]
        wdcp[64:128, dy, :] = wdc[:, dy * 3 + 1, :]
        wdcs[:, dy, :] = wdc[:, dy * 3 + 2, :]
    wdc_pack = np.zeros((128, 18), np.float32)
    wdc_pack[:, 0:9] = wdcp.reshape(128, 9)
    wdc_pack[0:64, 9:18] = wdcs.reshape(64, 9)
    out["wdc"] = wdc_pack.astype(f16)
    for Q in range(4):
        biases[32 * Q:32 * Q + 3, 12] = dcb
    out["biases"] = biases
    return out


def pos_encoding():
    c = np.arange(2, dtype=np.float32)
    yy = np.arange(64, dtype=np.float32)
    ang = yy[None, :] / (10000.0 ** (2.0 * c / 4.0)).astype(np.float32)[:, None]
    pe = np.zeros((4, 64), np.float32)
    pe[0::2] = np.sin(ang)
    pe[1::2] = np.cos(ang)
    return pe


def build_m0(x_shard, le_shard):
    """x_shard [ns,3,64,64] f32, le_shard [ns,64,64] f32 -> [ns,36,64,64] f16."""
    ns = x_shard.shape[0]
    pe = pos_encoding()
    h0 = np.zeros((ns, 4, 66, 66), np.float32)
    h0[:, :3, 1:65, 1:65] = x_shard
    h0[:, 3, 1:65, 1:65] = le_shard
    h0[:, :, 1:65, 1:65] += pe[None, :, :, None]
    m0 = np.zeros((ns, 36, 64, 64), np.float32)
    for dy in range(3):
        for dx in range(3):
            t = dy * 3 + dx
            m0[:, t * 4:t * 4 + 4] = h0[:, :, dy:dy + 64, dx:dx + 64]
    # permute columns so e1's relu write is contiguous in the h1d plane layout:
    # first 32 cols -> odd x (plane0 slots xx1..32), last 32 -> even x (plane1 xx0..31)
    m0p = np.empty_like(m0)
    m0p[:, :, :, 0:32] = m0[:, :, :, 1::2]
    m0p[:, :, :, 32:64] = m0[:, :, :, 0::2]
    return m0p.astype(np.float16)


def make_in_maps(x, labels, label_emb, static):
    le = label_emb[labels].reshape(-1, 64, 64)
    in_maps = []
    for c in range(NCORES):
        sl = slice(c * NS, (c + 1) * NS)
        m = dict(static)
        m["m0"] = build_m0(x[sl], le[sl])
        in_maps.append(m)
    return in_maps


def kernel(x, t, labels, label_emb, ew1, eb1, ew2, eb2, bw1, bb1,
           qw, qb, kw, kb, vw, vb, gamma, bw2, bb2, dtw, dtb, dcw, dcb):
    del t
    x = np.asarray(x, np.float32)
    labels = np.asarray(labels)
    label_emb = np.asarray(label_emb, np.float32)
    static = prep_static(np.asarray(ew1), np.asarray(eb1), np.asarray(ew2),
                         np.asarray(eb2), np.asarray(bw1), np.asarray(bb1),
                         np.asarray(qw), np.asarray(qb), np.asarray(kw),
                         np.asarray(kb), np.asarray(vw), np.asarray(vb),
                         np.asarray(gamma), np.asarray(bw2), np.asarray(bb2),
                         np.asarray(dtw), np.asarray(dtb), np.asarray(dcw),
                         np.asarray(dcb))
    in_maps = make_in_maps(x, labels, label_emb, static)
    if "nc" not in _cache:
        _cache["nc"] = build_nc()
    nc = _cache["nc"]
    res = run_bass_kernel_spmd(nc, in_maps, core_ids=list(range(NCORES)))
    return np.concatenate([res.results[c]["out"] for c in range(NCORES)], axis=0)



# revision 96
# speedup vs baseline: 2.0036x; 2.0036x over previous
"""Trainium2 Bass kernel for EnhancedConditionalUNet forward (B=64, 8 cores data-parallel).

Self-contained: hardcodes all shapes. kernel(**inputs) -> np.ndarray [64,3,64,64] f32.

v2 design (vs fp16 baseline, ~548us -> ~480us):
- attention block (q/k/v projections, scores, exp, colsum+replicate, attn-out)
  in fp8e4m3; DoubleRow packs 2 K-tiles per matmul (2x K per instruction at the
  same 1 col/cycle rate = 2x FLOPs); power-of-2 scale management (acts x2^5,
  attn weights x2^6) keeps evictions 2-op; colsum is fused with its row
  replication via a [128,2,128] ones lhsT DR matmul feeding one reciprocal
- conv backbone stays fp16 (fp8 fails the 2e-2 gate: each conv activation or
  weight site alone costs ~2.3e-2), but e2 and dc get physical tap-pairing: a
  shifted duplicate of the input activation on partitions 64-127 (SBUF->SBUF
  DMA) doubles K for the K<=64 layers
- dc interleaves its 4 tile_position column groups tap-major: narrow-M (M=3)
  matmuls in different <=32-wide column tiles overlap ~3x on the PE
- emission order interleaves sample s's serial attention chain with sample
  s-1's back-half convs so the PE never drains (p-state stays at 2.4GHz)

v3 (DMA/overlap round; timed via repeat-delta, profiled on CoreSim timeline):
- DMA cost is fixed-overhead dominated (~625ns global HWDGE hold + ~650-900ns
  latency per transfer), so: m0 loads merged 4->1 DMA per sample alternating
  SP/ACT queues; h1d dups merged 4->2; h5d dup merged 4->1 by padding rows to
  67 so one flat contiguous-run copy covers all rows (64 descriptors instead
  of 2048 strided ones); per-sample output DMAs moved to the gpsimd SWDGE
- all biases/scales packed into one [128,16] f32 table (one DMA, was 10)
- bordered activation tiles are persistent double-buffers (index s%2) and
  only their border cells are zeroed: the full-tile memsets serialized DVE
  for ~21us at startup, stalling e2's eviction and re-throttling the PE
- prev-sample dc matmul units interleaved between the scores/exp chunks of
  the current sample (engine FIFOs stall on PSUM-bank waits otherwise); dc
  tail restructured so output DMAs overlap the remaining tanh calls

v4 (sim span 443.6us -> 395.9us; rel err 1.32e-3):
- dc accumulation reordered singles-first: the wdcs single-tap matmuls read
  only partitions 0:64, so they run while the h5d dup (which every wdcp pair
  matmul needs) is still in flight — removed the dup from the critical path
  in both the steady state and the epilogue
- early scores chunks are filled with the NEXT sample's first four e1 rows
  (m0 is prefetched; evictions on DVE which is idle there), since dc units
  can't start until the dup lands; remaining dc units interleave from cc2
- softmax reciprocal via single-op reciprocal_approx_fast (18-bit accuracy,
  fine for denominators; halves the DVE recip cost and the at->b2 chain)
- last two samples' output DMAs on the HWDGE queues (the single SWDGE ring
  serialized 4x1.6us after the final matmul); s0 h1d dups split per-r on the
  idle Pool queue so e2(s0) starts ~2.5us earlier
- NOTE: fp8 for the conv backbone is a dead end at this 2e-2 gate — fp8
  dot-product noise is ~3-6% of the OUTPUT independent of K (noise terms
  scale with the same terms that form the sum), only the gamma=0.1-suppressed
  attention path tolerates it. Engine moves that put work on DVE during the
  rep->recip->t1->hb2 window, or evictions on Pool (0.42 eff), regressed.

v5 (sim span 395.9us -> 388.0us; DVE-backlog round):
- qkv moved back before dt[2,3]: the v-projection PSUM evictions (DVE) now
  overlap dt's PE work instead of stalling the v matmuls on psS recycling
- early-emitted e1 rows evict on DVE (ACT is saturated by exp there; gpsimd
  is ILLEGAL for this — GPSIMD cannot access PSUM, the BIR verifier rejects
  the NEFF even though CoreSim happily simulates it)
- e2 r=0 emitted right after rep_at (its inputs exist from the early e1
  rows + first dup) with ACT evictions: covers b2's 1us wait on the
  recip->t1->hb2 DVE chain without lengthening that chain
- last-two-samples' output DMAs split sync/Pool (never the ACT queue: the
  DMA seq slot would sit between the final tanh calls and delay them)

v6 (sim span 389.1us -> 382.7us):
- v-projection evictions on ACT via AF.Copy (identity-with-scale; ACT may
  read PSUM, gpsimd may not): the DVE backlog behind the fast DR matmul
  pairs was stalling psS recycling. All-ACT beat an ACT/DVE alternation.
- s0's S phase (no prev-dc units) now absorbs all 8 of s1's e1 rows
- tried and reverted: splitting s0's m0 load / moving s0 dups to sync
  (queue congestion + longer HAM cold phase), alternating v evictions.
v7: PE warm-up — 22 dummy fp8 matmuls on the ones8 constant fill the
otherwise-idle window before the first m0 DMA lands, so the HAM clock gate
reaches 2.4GHz before the real e1 matmuls start (sim-span-neutral: the
startup critical path is DMA latency; on HW it removes the cold-clock
first-sample penalty). S-phase row-tiling was costed and rejected: the k/q
replication DMAs (+5us/sample of queue time) exceed the ~2.5us of HW
column-overlap they would buy.

v8 (sim span 382.7us -> 381.5us; final): sample 0's e2 runs in SPLIT form —
tap (dy,1) is read from plane 1 of the original partitions via a separate
K=64 matmul against the wepb weight copy (lhsT/rhs must share a base
partition). Three extra matmuls per r, but no dependency on the h1d dup
DMAs, so s0 emits no dups at all and e2(s0) starts ~2.5us earlier.

v9 (sim span 381.5us -> 377.8us): dc reduced from 6 to 5 K-passes — a
second, y+1-shifted duplicate of h5 (h5y: [0:64] verbatim, [64:128] +1 row,
two contiguous flat-run DMAs on sync) merges the (0,2)+(1,2) single taps
into one K=128 pair pass (48 -> 40 matmuls/sample, -11.8us PE total). The
last sample keeps the legacy 6-pass singles-first form: with only one
dup-free pass the epilogue re-exposed the dup latency (~4.3us) that the
steady-state S-phase interleave hides. Routing either h5y copy via the ACT
queue regressed (DMA seq slot collides with exp/tanh).

Remaining known floor: ~6.8us final drain (4 serial tanh + out DMA latency),
~5us one-time startup pipeline-fill edges, and the structural fp16 PE work
(PE busy ~90% at the reduced work level).
"""
import numpy as np
import ml_dtypes

import bass_rust as _br
import concourse.bass as bass
import concourse.tile as tile
from concourse import bacc, mybir
from concourse.bass_utils import run_bass_kernel_spmd

NCORES = 8
NS = 8          # samples per core
BF = mybir.dt.bfloat16
F16 = mybir.dt.float16
F32 = mybir.dt.float32
F8 = mybir.dt.float8e4
NPF8 = ml_dtypes.float8_e4m3
AF = mybir.ActivationFunctionType
OP = mybir.AluOpType
DR = mybir.MatmulPerfMode.DoubleRow

_cache = {}


def build_nc(ns=NS, repeat=1):
    nc = bacc.Bacc("TRN2", target_bir_lowering=False, debug=False)

    d_m0 = nc.dram_tensor("m0", [ns, 36, 64, 64], F16, kind="ExternalInput")
    d_wim = nc.dram_tensor("wim", [36, 64], F16, kind="ExternalInput")
    d_wep = nc.dram_tensor("wep", [128, 3, 128], F16, kind="ExternalInput")
    d_wes = nc.dram_tensor("wes", [64, 3, 128], F16, kind="ExternalInput")
    d_wepb = nc.dram_tensor("wepb", [64, 3, 128], F16, kind="ExternalInput")
    d_wb1 = nc.dram_tensor("wb1", [128, 9, 2, 128], F16, kind="ExternalInput")
    d_wqk8 = nc.dram_tensor("wqk8", [128, 2, 64], F8, kind="ExternalInput")
    d_wv8 = nc.dram_tensor("wv8", [128, 2, 256], F8, kind="ExternalInput")
    d_wb2 = nc.dram_tensor("wb2", [128, 2, 9, 128], F16, kind="ExternalInput")
    d_wdt = nc.dram_tensor("wdt", [128, 16, 64], F16, kind="ExternalInput")
    d_wdc = nc.dram_tensor("wdc", [128, 21], F16, kind="ExternalInput")
    d_bias = nc.dram_tensor("biases", [128, 16], F32, kind="ExternalInput")
    d_out = nc.dram_tensor("out", [ns, 3, 64, 64], F32, kind="ExternalOutput")

    with tile.TileContext(nc) as tc:
        with (
            tc.tile_pool(name="wpool", bufs=1) as wp,
            tc.tile_pool(name="apool", bufs=2) as ap,
            tc.tile_pool(name="spool", bufs=1) as sp,
            tc.tile_pool(name="psS", bufs=4, space="PSUM") as psS,
            tc.tile_pool(name="psB", bufs=2, space="PSUM") as psB,
        ):
            def wload(name, shape, dt, dram, eng):
                t = wp.tile(shape, dt, name=name)
                eng.dma_start(t[:], dram[:])
                return t

            _m0pre = {}

            def m0_load(s):
                m0 = ap.tile([36, 64, 64], F16, name="m0t", bufs=3)
                eng = nc.sync if s % 2 == 0 else nc.scalar
                eng.dma_start(m0[:, :, :], d_m0[s])
                return m0

            wim = wload("wim", [36, 64], F16, d_wim, nc.scalar)
            biasT = wload("biasT", [128, 16], F32, d_bias, nc.scalar)
            _m0pre[0] = m0_load(0)
            wep = wload("wep", [128, 3, 128], F16, d_wep, nc.scalar)
            wes = wload("wes", [64, 3, 128], F16, d_wes, nc.scalar)
            wepb = wload("wepb", [64, 3, 128], F16, d_wepb, nc.scalar)
            wb1 = wload("wb1", [128, 9, 2, 128], F16, d_wb1, nc.sync)
            wqk8 = wload("wqk8", [128, 2, 64], F8, d_wqk8, nc.gpsimd)
            wv8 = wload("wv8", [128, 2, 256], F8, d_wv8, nc.gpsimd)
            ones8 = wp.tile([128, 2, 128], F8)
            nc.vector.memset(ones8[:], 1.0)

            # PE warm-up: the HAM clock gate holds the PE at half clock until
            # ~3.4us of sustained activity. The PE would idle until the first
            # m0 DMA lands (~6us) anyway, so burn dummy matmuls on the ones8
            # constant; the real e1/e2 matmuls then start at full clock.
            warm_ps = psS.tile([128, 512], F32, name="pcs")
            ones_r = ones8.rearrange("p a b -> p (a b)")
            for _ in range(22):
                nc.tensor.matmul(warm_ps[:, 0:256], ones_r[:, 0:128],
                                 ones_r[:], start=True, stop=True)

            E = ap.tile([128, 8, 1024], F8, name="E", bufs=1)
            invg = sp.tile([128, 1024], F32, name="invg")
            scr = sp.tile([128, 1024], F32, name="scr")
            t1 = sp.tile([128, 1024], F32, name="t1")

            def zero_all(t):
                nc.vector.memset(t[:], 0.0)

            taps = [(dy, dx) for dy in range(3) for dx in range(3)]

            def f_e1(c, rs, evict_dve=False):
                """e1: im2col matmuls -> relu -> h1d[:64]; dup [64:] via DMA."""
                s, h1d = c["s"], c["h1d"]
                if rs[0] == 0:
                    m0 = _m0pre.pop(s, None)
                    if m0 is None:
                        m0 = m0_load(s)
                    c["m0"] = m0
                m0 = c["m0"]
                h1f = h1d.rearrange("p a b c -> p a (b c)")
                h1flat = h1d.rearrange("p a b c -> p (a b c)")
                for r in rs:
                    ps = psS.tile([128, 512], F32, name="pcs")
                    nc.tensor.matmul(ps[0:64], wim[:], m0[:, 8 * r:8 * r + 8, :],
                                     start=True, stop=True)
                    if evict_dve or (s == 0 and r % 2 == 1):
                        # DVE eviction where ACT is contended (exp phase /
                        # s=0 startup). NOTE: gpsimd is NOT an option here —
                        # GPSIMD instructions cannot access PSUM (BIR
                        # verifier rejects it; CoreSim doesn't check).
                        nc.vector.tensor_scalar(
                            out=h1f[0:64, 1 + 8 * r:9 + 8 * r, 1:65],
                            in0=ps[0:64].rearrange("p (a b) -> p a b", a=8),
                            scalar1=biasT[0:64, 0:1], scalar2=0.0,
                            op0=OP.add, op1=OP.max)
                    else:
                        c["relu_last"] = nc.scalar.activation(
                            h1f[0:64, 1 + 8 * r:9 + 8 * r, 1:65],
                            ps[0:64].rearrange("p (a b) -> p a b", a=8),
                            AF.Relu, bias=biasT[0:64, 0:1], scale=1.0)
                    if s > 0 and r in (3, 7):
                        # two merged dups (fewer DMAs win). s=0 needs none:
                        # its e2 runs in split form (no dup partitions)
                        o0, nn = (66, 2079) if r == 3 else (33 * 66, 2112)
                        nc.gpsimd.dma_start(
                            h1flat[64:128, o0:o0 + nn],
                            h1flat[0:64, o0 + 33:o0 + 33 + nn])

            def f_e2(c, rr, split=False):
                """split=True (sample 0): tap (dy,1) is read from plane 1 of
                the original partitions as a separate K=64 matmul — 3 extra
                matmuls, but no dependency on the h1d dup DMAs at all."""
                h1d, h2 = c["h1d"], c["h2"]
                for r in rr:
                    ps = psS.tile([128, 512], F32, name="pcs")
                    n = 0
                    for dy in range(3):
                        if split:
                            nc.tensor.matmul(
                                ps[:], wep[0:64, dy, :],
                                h1d[0:64, dy + 32 * r:dy + 32 * r + 32:2, 0, 0:32],
                                start=(n == 0), stop=False)
                            n += 1
                            nc.tensor.matmul(
                                ps[:], wepb[:, dy, :],
                                h1d[0:64, dy + 32 * r:dy + 32 * r + 32:2, 1, 0:32],
                                start=False, stop=False)
                            n += 1
                        else:
                            nc.tensor.matmul(
                                ps[:], wep[:, dy, :],
                                h1d[:, dy + 32 * r:dy + 32 * r + 32:2, 0, 0:32],
                                start=(n == 0), stop=False)
                            n += 1
                    nlast = n + 3
                    for dy in range(3):
                        n += 1
                        nc.tensor.matmul(ps[:], wes[:, dy, :],
                                         h1d[0:64, dy + 32 * r:dy + 32 * r + 32:2, 0, 1:33],
                                         start=False, stop=(n == nlast))
                    # ACT eviction: e2 r=0 is emitted inside the attention
                    # window to cover b2's wait on the DVE chain — a DVE
                    # eviction here would lengthen that very chain
                    nc.scalar.activation(h2[:, 1 + 16 * r:17 + 16 * r, 1:33],
                                         ps[:], AF.Relu, bias=biasT[:, 1:2],
                                         scale=1.0)

            def f_b1(c):
                h2 = c["h2"]
                h3f16, h3f8 = c["h3f16"], c["h3f8"]
                for mh in range(2):
                    for r in range(2):
                        ps = psS.tile([128, 512], F32, name="pcs")
                        for ti, (dy, dx) in enumerate(taps):
                            nc.tensor.matmul(
                                ps[:], wb1[:, ti, mh, :],
                                h2[:, dy + 16 * r:dy + 16 * r + 16, dx:dx + 32],
                                start=(ti == 0), stop=(ti == 8))
                        nc.scalar.activation(
                            h3f8[:, mh, 512 * r:512 * r + 512], ps[:],
                            AF.Relu, bias=biasT[:, 4 + mh:5 + mh], scale=32.0)
                        nc.vector.tensor_scalar(
                            out=h3f16[:, mh, 512 * r:512 * r + 512],
                            in0=ps[:], scalar1=biasT[:, 2 + mh:3 + mh],
                            scalar2=0.0, op0=OP.add, op1=OP.max)

            def f_qkv(c):
                """qk fused DR matmul + k relayout DMA; v DR matmuls -> vT8."""
                h3f8, qk8, ksb, vT8 = c["h3f8"], c["qk8"], c["ksb"], c["vT8"]
                for r in range(2):
                    ps = psS.tile([128, 512], F32, name="pcs")
                    nc.tensor.matmul(ps[0:64], wqk8[:], h3f8[:, :, 512 * r:512 * r + 512],
                                     start=True, stop=True, perf_mode=DR)
                    nc.vector.tensor_scalar(out=qk8[:, 512 * r:512 * r + 512],
                                            in0=ps[0:64], scalar1=biasT[0:64, 6:7],
                                            scalar2=2.0 ** -5, op0=OP.add, op1=OP.mult)
                    nc.gpsimd.dma_start(ksb[:, 512 * r:512 * r + 512],
                                        qk8[32:64, 512 * r:512 * r + 512])
                for cc2 in range(4):
                    ps = psS.tile([128, 512], F32, name="pcs")
                    for u in range(2):
                        cc = 2 * cc2 + u
                        nc.tensor.matmul(ps[:, 256 * u:256 * u + 256],
                                         h3f8[:, :, 128 * cc:128 * cc + 128],
                                         wv8[:], start=True, stop=True, perf_mode=DR)
                    # ACT Copy (identity with scale): keeps the v evictions
                    # off DVE, whose backlog stalls psS recycling here
                    c["v_last"] = nc.scalar.activation(
                        vT8[:, 2 * cc2:2 * cc2 + 2, :],
                        ps[:], AF.Copy, scale=2.0 ** -6)

            def f_S(c, pc=None, defer=0, nxt=None):
                """scores (fp8, K=32) + exp -> E fp8. Fillers between chunks:
                cc1-2 emit the NEXT sample's first e1 rows (the h5d dup that
                gates every dc unit is still in flight then); cc3+ interleave
                prev-sample dc units. Returns `defer` unemitted units."""
                qk8, ksb, E = c["qk8"], c["ksb"], c["E"]
                units = dc_units(pc) if pc is not None else []
                stop = len(units) - defer
                ui = 0
                for cc in range(8):
                    sps = psB.tile([128, 1024], F32, name="pbig")
                    for ih in range(2):
                        nc.tensor.matmul(sps[:, 512 * ih:512 * ih + 512],
                                         ksb[:, 128 * cc:128 * cc + 128],
                                         qk8[0:32, 512 * ih:512 * ih + 512],
                                         start=True, stop=True)
                    nc.scalar.activation(E[:, cc, :], sps[:], AF.Exp, scale=2.0 ** -12)
                    if nxt is not None and (cc in (1, 2) or
                                            (pc is None and cc in (3, 4))):
                        # s0's S phase has no prev-dc units: emit all 8 of
                        # the next sample's e1 rows there instead of 4
                        f_e1(nxt, [2 * cc - 2, 2 * cc - 1], evict_dve=True)
                    take = {0: 0, 1: 0, 2: 1, 7: 1}.get(cc, 2)
                    for _ in range(take):
                        if ui < stop:
                            units[ui]()
                            ui += 1
                while ui < stop:
                    units[ui]()
                    ui += 1
                return units[stop:]

            def f_rep_at(c):
                """colsum-rep, reciprocal, attn out, normalize -> hb2."""
                E, vT8, h3f16, hb2 = c["E"], c["vT8"], c["h3f16"], c["hb2"]
                rep = psB.tile([128, 1024], F32, name="pbig")
                for ih in range(2):
                    for cp in range(4):
                        nc.tensor.matmul(rep[:, 512 * ih:512 * ih + 512], ones8[:],
                                         E[:, 2 * cp:2 * cp + 2, 512 * ih:512 * ih + 512],
                                         start=(cp == 0), stop=(cp == 3), perf_mode=DR)
                nc.vector.reciprocal_approx_fast(invg[:], rep[:])
                for ch in range(2):
                    at = psB.tile([128, 1024], F32, name="pbig")
                    for ih in range(2):
                        for cp in range(4):
                            nc.tensor.matmul(
                                at[:, 512 * ih:512 * ih + 512],
                                vT8[:, 2 * cp:2 * cp + 2, 128 * ch:128 * ch + 128],
                                E[:, 2 * cp:2 * cp + 2, 512 * ih:512 * ih + 512],
                                start=(cp == 0), stop=(cp == 3), perf_mode=DR)
                    nc.vector.scalar_tensor_tensor(
                        out=t1[:], in0=at[:], scalar=biasT[:, 9:10], in1=invg[:],
                        op0=OP.mult, op1=OP.mult)
                    nc.vector.scalar_tensor_tensor(
                        out=hb2[:, ch, 1:33, 1:33],
                        in0=t1[:], scalar=biasT[:, 7 + ch:8 + ch],
                        in1=h3f16[:, ch, :], op0=OP.add, op1=OP.add)

            def f_b2(c):
                hb2, h4 = c["hb2"], c["h4"]
                wb2 = late["wb2"]
                for r in range(2):
                    ps = psS.tile([128, 512], F32, name="pcs")
                    for kh in range(2):
                        for ti, (dy, dx) in enumerate(taps):
                            nc.tensor.matmul(
                                ps[:], wb2[:, kh, ti, :],
                                hb2[:, kh, dy + 16 * r:dy + 16 * r + 16, dx:dx + 32],
                                start=(kh == 0 and ti == 0), stop=(kh == 1 and ti == 8))
                    nc.vector.tensor_scalar(out=h4[:, 1 + 16 * r:17 + 16 * r, 1:33],
                                            in0=ps[:], scalar1=biasT[:, 10:11],
                                            scalar2=0.0, op0=OP.add, op1=OP.max)

            def f_dt(c, phases):
                h4, h5d = c["h4"], c["h5d"]
                wdt = late["wdt"]
                aoff = {(0, 0): 1, (0, 1): 0, (1, 0): 2, (1, 1): 1}
                for ph in phases:
                    py, px = ph // 2, ph % 2
                    if True:
                        for r in range(2):
                            ps = psS.tile([128, 512], F32, name="pcs")
                            ti = 0
                            for dy2 in range(2):
                                for dx2 in range(2):
                                    ay = aoff[(py, dy2)]
                                    ax = aoff[(px, dx2)]
                                    nc.tensor.matmul(
                                        ps[0:64], wdt[:, ph * 4 + dy2 * 2 + dx2, :],
                                        h4[:, ay + 16 * r:ay + 16 * r + 16, ax:ax + 32],
                                        start=(ti == 0), stop=(ti == 3))
                                    ti += 1
                            y0 = 1 + py + 32 * r
                            out_ap = h5d[0:64, y0:y0 + 32:2, 1 + px:1 + px + 64:2]
                            if py == 0:
                                nc.vector.tensor_scalar(out=out_ap, in0=ps[0:64],
                                                        scalar1=biasT[0:64, 11:12],
                                                        scalar2=0.0,
                                                        op0=OP.add, op1=OP.max)
                            else:
                                nc.scalar.activation(out_ap, ps[0:64], AF.Relu,
                                                     bias=biasT[0:64, 11:12],
                                                     scale=1.0)
                    if ph == 3:
                        # all 4 phases written: one contiguous flat-range dup
                        # (row width padded to 67 so runs span row boundaries)
                        deng = nc.gpsimd if c["s"] == ns - 1 else nc.sync
                        h5f = h5d.rearrange("p a b -> p (a b)")
                        deng.dma_start(h5f[64:128, 67:67 + 4288],
                                       h5f[0:64, 68:68 + 4288])
                        # y-pair buffer: [0:64] = h5 verbatim, [64:128] =
                        # h5 shifted one row (+67 flat) — lets dc pair taps
                        # (0,2)+(1,2) into one K=128 pass (6 passes -> 5)
                        if c["s"] < ns - 1:
                            h5yf = c["h5y"].rearrange("p a b -> p (a b)")
                            nc.sync.dma_start(h5yf[0:64, 0:4422],
                                              h5f[0:64, 0:4422])
                            nc.sync.dma_start(h5yf[64:128, 0:4355],
                                              h5f[0:64, 67:4422])

            def dc_units(c):
                """dc as a list of emit-closures so f_S can interleave them."""
                s, h5d, dct = c["s"], c["h5d"], c["dct"]
                wdc = late["wdc"]
                _ = c["h5y"]
                units = []
                qts = {}

                # last sample: legacy 6-pass form (3 dup-free single
                # passes cover the dup latency in the epilogue, where there
                # is no other PE work left to interleave)
                legacy = (s == ns - 1)

                def mk_mm(r2, n):
                    # 5-pass order per r2: n=0 single (2,2) (dup-free, K=64),
                    # n=1..3 x-pairs (h5d dup), n=4 y-pair (h5y bufs, K=128:
                    # taps (0,2)+(1,2) merged). tap-major within each pass:
                    # consecutive matmuls hit different PE column tiles.
                    def emit():
                        if n == 0:
                            qts[r2] = psS.tile([128, 512], F32, name="pcs")
                        qt = qts[r2]
                        nsing = 3 if legacy else 1
                        for Q in range(4):
                            y0 = 16 * Q + 8 * r2
                            if n < nsing:
                                dy = 2 - n if not legacy else n
                                nc.tensor.matmul(
                                    qt[32 * Q:32 * Q + 3, :],
                                    wdc[0:64, 9 + 3 * dy:12 + 3 * dy],
                                    h5d[0:64, dy + y0:dy + y0 + 8, 2:66],
                                    start=(n == 0), stop=False,
                                    tile_position=(0, 32 * Q))
                            elif n < nsing + 3:
                                dy = n - nsing
                                nc.tensor.matmul(
                                    qt[32 * Q:32 * Q + 3, :],
                                    wdc[:, 3 * dy:3 * dy + 3],
                                    h5d[:, dy + y0:dy + y0 + 8, 0:64],
                                    start=False,
                                    stop=(legacy and n == nsing + 2),
                                    tile_position=(0, 32 * Q))
                            else:
                                nc.tensor.matmul(
                                    qt[32 * Q:32 * Q + 3, :],
                                    wdc[:, 18:21],
                                    c["h5y"][:, y0:y0 + 8, 2:66],
                                    start=False, stop=True,
                                    tile_position=(0, 32 * Q))
                    return emit

                def _depri(inst):
                    # nosync (scheduling-only) edge: run the tanh after the
                    # next-next sample's e1 relu chain — its ACT calls would
                    # otherwise collide with e1/e2 evictions in the post-rep
                    # window and stall the PE's psS recycling (827ns x6).
                    # Last sample excluded: nothing competes in the epilogue.
                    d = dep_box.get("i")
                    if d is not None and s < ns - 1:
                        inst.ins.add_nosync_dependencies_from(
                            _br.InstructionNameOrderedSet([d.ins.name]))

                def mk_tanh(r2):
                    def emit():
                        qt = qts[r2]
                        for Q in range(4):
                            _depri(nc.scalar.activation(
                                dct[32 * Q:32 * Q + 3, 512 * r2:512 * r2 + 512],
                                qt[32 * Q:32 * Q + 3, :], AF.Tanh,
                                bias=biasT[32 * Q:32 * Q + 3, 12:13], scale=1.0))
                    return emit

                def mk_tanhout(Q):
                    # r2=1 tanh for this Q, then its output DMA — so the out
                    # DMAs overlap the remaining tanh calls at the tail. The
                    # last two samples' outputs go on the HWDGE queues (idle
                    # at the end; the single SWDGE ring would serialize them
                    # after the final matmul).
                    def emit():
                        qt = qts[1]
                        _depri(nc.scalar.activation(
                            dct[32 * Q:32 * Q + 3, 512:1024],
                            qt[32 * Q:32 * Q + 3, :], AF.Tanh,
                            bias=biasT[32 * Q:32 * Q + 3, 12:13], scale=1.0))
                        # sync/Pool mix (not scalar: a DMA on the ACT queue
                        # would sit between the final tanh calls): two idle
                        # queues drain the last outputs in parallel
                        if s >= ns - 2:
                            deng = nc.sync if Q % 2 == 0 else nc.gpsimd
                        else:
                            deng = nc.gpsimd
                        deng.dma_start(
                            d_out[s][:, 16 * Q:16 * Q + 16, :],
                            dct[32 * Q:32 * Q + 3, :])
                    return emit

                npass = 6 if legacy else 5
                for n in range(npass):
                    units.append(mk_mm(0, n))
                units.append(mk_tanh(0))
                for n in range(npass):
                    units.append(mk_mm(1, n))
                for Q in range(4):
                    units.append(mk_tanhout(Q))
                return units

            def f_dc(c):
                for u in dc_units(c):
                    u()

            # bordered tiles: persistent double-buffers (borders stay zero
            # across generations; single tensor id keeps the sim shadow happy)
            perst = {}
            for nm, shape in (("h1d", [128, 66, 2, 33]), ("h2", [128, 34, 34]),
                              ("hb2", [128, 2, 34, 34]), ("h4", [128, 34, 34]),
                              ("h5d", [128, 66, 67]),
                              ("h5y", [128, 66, 67])):
                perst[nm] = [sp.tile(shape, F16, name=f"{nm}_{i}")
                             for i in range(2)]

            def alloc_ctx(s):
                return dict(
                    s=s,
                    h1d=perst["h1d"][s % 2],
                    h2=perst["h2"][s % 2],
                    h3f16=ap.tile([128, 2, 1024], F16, name="h3f16"),
                    h3f8=ap.tile([128, 2, 1024], F8, name="h3f8"),
                    qk8=ap.tile([64, 1024], F8, name="qk8"),
                    ksb=ap.tile([32, 1024], F8, name="ksb"),
                    vT8=ap.tile([128, 8, 256], F8, name="vT8"),
                    hb2=perst["hb2"][s % 2],
                    h4=perst["h4"][s % 2],
                    h5d=perst["h5d"][s % 2],
                    h5y=perst["h5y"][s % 2],
                    dct=ap.tile([128, 1024], F32, name="dct"),
                    E=E,
                )

            late = {}
            prev = None
            pending = []
            dep_box = {}
            # emission order interleaves sample s's serial attention chain with
            # sample s-1's back-half conv matmuls so the PE never drains
            # zero only the border cells the convs read (rows/cols outside the
            # written interior) — full-tile memsets serialized DVE for ~21us
            for i in range(2):
                h1d_, h2_, hb2_, h4_, h5d_ = (perst[n][i] for n in
                                              ("h1d", "h2", "hb2", "h4", "h5d"))
                V, G = nc.vector, nc.gpsimd
                V.memset(h1d_[:, 0], 0.0)
                V.memset(h1d_[:, 65], 0.0)
                V.memset(h1d_[:, :, 0, 0:1], 0.0)
                V.memset(h1d_[:, :, 1, 32:33], 0.0)
                for t in (h2_, h4_):
                    G.memset(t[:, 0], 0.0)
                    G.memset(t[:, 33], 0.0)
                    G.memset(t[:, :, 0:1], 0.0)
                    G.memset(t[:, :, 33:34], 0.0)
                G.memset(hb2_[:, :, 0], 0.0)
                G.memset(hb2_[:, :, 33], 0.0)
                G.memset(hb2_[:, :, :, 0:1], 0.0)
                G.memset(hb2_[:, :, :, 33:34], 0.0)
                V.memset(h5d_[:, 0], 0.0)
                V.memset(h5d_[:, 65], 0.0)
                V.memset(h5d_[:, :, 0:1], 0.0)
                V.memset(h5d_[:, :, 65:67], 0.0)
            nxt_box = {}
            for rep in range(repeat):
              for s in range(ns):
                c = nxt_box.pop("c", None)
                pre_rs = nxt_box.pop("pre", 0) if c is not None else 0
                if c is None:
                    c = alloc_ctx(s)
                if prev is not None:
                    f_rep_at(prev)
                if pre_rs >= 4:
                    # rows 0-3 + the first dup were emitted inside the prev
                    # sample's S phase, so e2 r=0 is ready now: it covers
                    # b2's wait on the attention DVE chain (recip/t1/hb2)
                    f_e2(c, [0])
                    if pre_rs < 8:
                        f_e1(c, [4, 5, 6, 7])
                else:
                    f_e1(c, [0, 1, 2, 3, 4, 5, 6, 7])
                split0 = (pre_rs == 0)
                if rep == 0 and s == 0:
                    late["wb2"] = wload("wb2", [128, 2, 9, 128], F16, d_wb2,
                                        nc.sync)
                    late["wdt"] = wload("wdt", [128, 16, 64], F16, d_wdt,
                                        nc.scalar)
                    late["wdc"] = wload("wdc", [128, 21], F16, d_wdc, nc.gpsimd)
                if prev is not None:
                    f_b2(prev)
                # deferred tanh+out units of the sample before prev: their
                # ACT calls would otherwise collide with e1/e2 evictions in
                # the post-rep_at window (b1's window has ACT slack)
                dep_box["i"] = c.get("relu_last")
                for u in pending:
                    u()
                pending = []
                dep_box["i"] = None
                if pre_rs >= 4:
                    f_e2(c, [1])
                else:
                    f_e2(c, [0, 1], split=split0)
                f_b1(c)
                if prev is not None:
                    f_dt(prev, [0, 1])
                f_qkv(c)
                if prev is not None:
                    f_dt(prev, [2, 3])
                last = (rep == repeat - 1 and s == ns - 1)
                nxt = None
                if s + 1 < ns:
                    nxt = alloc_ctx(s + 1)
                    nxt_box["c"] = nxt
                    nxt_box["pre"] = 8 if prev is None else 4
                pending = f_S(c, prev, defer=8 if last else 4, nxt=nxt)
                prev = c
            if prev is not None:
                f_rep_at(prev)
                f_b2(prev)
                f_dt(prev, [0, 1, 2, 3])
                for u in pending:
                    u()
                f_dc(prev)

    nc.compile()
    return nc


def prep_static(ew1, eb1, ew2, eb2, bw1, bb1, qw, qb, kw, kb, vw, vb,
                gamma, bw2, bb2, dtw, dtb, dcw, dcb):
    """Host-side weight layout prep (shared across cores)."""
    f16 = np.float16
    f32 = np.float32
    out = {}
    wim = np.zeros((36, 64), np.float32)
    for dy in range(3):
        for dx in range(3):
            t = dy * 3 + dx
            wim[t * 4:t * 4 + 4, :] = ew1[:, :, dy, dx].T
    out["wim"] = wim.astype(f16)
    # packed per-partition bias/scale table [128, 16] f32:
    # col 0: be1(p0:64) | 1: be2 | 2-3: bb1 | 4-5: bb1s | 6: bqk(p0:64)
    # 7-8: gvb | 9: gam5 | 10: bb2 | 11: bdt(p0:64) | 12: bdc
    biases = np.zeros((128, 16), f32)
    biases[0:64, 0] = eb1
    # e2: paired taps (dy,0)+(dy,1) stacked on K, singles (dy,2)
    we2 = np.transpose(ew2, (1, 2, 3, 0)).reshape(64, 9, 128)  # [cin, tap, cout]
    wep = np.zeros((128, 3, 128), np.float32)
    wes = np.zeros((64, 3, 128), np.float32)
    for dy in range(3):
        wep[0:64, dy, :] = we2[:, dy * 3 + 0, :]
        wep[64:128, dy, :] = we2[:, dy * 3 + 1, :]
        wes[:, dy, :] = we2[:, dy * 3 + 2, :]
    out["wep"] = wep.astype(f16)
    out["wepb"] = wep[64:128].astype(f16)
    out["wes"] = wes.astype(f16)
    biases[:, 1] = eb2
    wb1 = np.transpose(bw1, (1, 2, 3, 0)).reshape(128, 9, 2, 128)
    out["wb1"] = np.ascontiguousarray(wb1).astype(f16)
    biases[:, 2:4] = bb1.reshape(2, 128).T
    biases[:, 4:6] = biases[:, 2:4] * 32.0
    # attention fp8: wqk8 [128(kp),2(kh),64(m)] = [q cols 0:32 | k cols 32:64] x2^6
    wq_t = qw[:, :, 0, 0].T.reshape(2, 128, 32)   # [kh, kp, c]
    wk_t = kw[:, :, 0, 0].T.reshape(2, 128, 32)
    wqk8 = np.zeros((128, 2, 64), np.float32)
    wqk8[:, :, 0:32] = wq_t.transpose(1, 0, 2) * 64.0
    wqk8[:, :, 32:64] = wk_t.transpose(1, 0, 2) * 64.0
    out["wqk8"] = wqk8.astype(NPF8)
    biases[0:64, 6] = np.concatenate([qb, kb]) * 2.0 ** 11
    wv = vw[:, :, 0, 0].T.reshape(2, 128, 256).transpose(1, 0, 2)
    out["wv8"] = np.ascontiguousarray(wv * 64.0).astype(NPF8)
    g = float(np.asarray(gamma).reshape(-1)[0])
    biases[:, 7:9] = (g * vb).reshape(2, 128).T
    biases[:, 9] = g * 2.0 ** -5
    wb2_ = np.transpose(bw2, (1, 2, 3, 0)).reshape(2, 128, 9, 128).transpose(1, 0, 2, 3)
    out["wb2"] = np.ascontiguousarray(wb2_).astype(f16)
    biases[:, 10] = bb2
    kmap = {(0, 0): 1, (0, 1): 3, (1, 0): 0, (1, 1): 2}
    wdt = np.zeros((128, 16, 64), np.float32)
    for py in range(2):
        for px in range(2):
            for dy2 in range(2):
                for dx2 in range(2):
                    ky = kmap[(py, dy2)]
                    kx = kmap[(px, dx2)]
                    wdt[:, (py * 2 + px) * 4 + dy2 * 2 + dx2, :] = dtw[:, :, ky, kx]
    out["wdt"] = wdt.astype(f16)
    biases[0:64, 11] = dtb
    # dc: paired taps (dy,0)+(dy,1) on K, singles (dy,2)
    wdc = np.transpose(dcw, (1, 2, 3, 0)).reshape(64, 9, 3)  # [cin, tap, cout]
    wdcp = np.zeros((128, 3, 3), np.float32)
    wdcs = np.zeros((64, 3, 3), np.float32)
    for dy in range(3):
        wdcp[0:64, dy, :] = wdc[:, dy * 3 + 0, :]
        wdcp[64:128, dy, :] = wdc[:, dy * 3 + 1, :]
        wdcs[:, dy, :] = wdc[:, dy * 3 + 2, :]
    wdc_pack = np.zeros((128, 21), np.float32)
    wdc_pack[:, 0:9] = wdcp.reshape(128, 9)
    wdc_pack[0:64, 9:18] = wdcs.reshape(64, 9)
    # y-pair chunk: partitions 0:63 = tap (0,2) chans, 64:127 = tap (1,2)
    wdc_pack[0:64, 18:21] = wdc[:, 2, :]
    wdc_pack[64:128, 18:21] = wdc[:, 5, :]
    out["wdc"] = wdc_pack.astype(f16)
    for Q in range(4):
        biases[32 * Q:32 * Q + 3, 12] = dcb
    out["biases"] = biases
    return out


def pos_encoding():
    c = np.arange(2, dtype=np.float32)
    yy = np.arange(64, dtype=np.float32)
    ang = yy[None, :] / (10000.0 ** (2.0 * c / 4.0)).astype(np.float32)[:, None]
    pe = np.zeros((4, 64), np.float32)
    pe[0::2] = np.sin(ang)
    pe[1::2] = np.cos(ang)
    return pe


def build_m0(x_shard, le_shard):
    """x_shard [ns,3,64,64] f32, le_shard [ns,64,64] f32 -> [ns,36,64,64] f16."""
    ns = x_shard.shape[0]
    pe = pos_encoding()
    h0 = np.zeros((ns, 4, 66, 66), np.float32)
    h0[:, :3, 1:65, 1:65] = x_shard
    h0[:, 3, 1:65, 1:65] = le_shard
    h0[:, :, 1:65, 1:65] += pe[None, :, :, None]
    m0 = np.zeros((ns, 36, 64, 64), np.float32)
    for dy in range(3):
        for dx in range(3):
            t = dy * 3 + dx
            m0[:, t * 4:t * 4 + 4] = h0[:, :, dy:dy + 64, dx:dx + 64]
    # permute columns so e1's relu write is contiguous in the h1d plane layout:
    # first 32 cols -> odd x (plane0 slots xx1..32), last 32 -> even x (plane1 xx0..31)
    m0p = np.empty_like(m0)
    m0p[:, :, :, 0:32] = m0[:, :, :, 1::2]
    m0p[:, :, :, 32:64] = m0[:, :, :, 0::2]
    return m0p.astype(np.float16)


def make_in_maps(x, labels, label_emb, static):
    le = label_emb[labels].reshape(-1, 64, 64)
    in_maps = []
    for c in range(NCORES):
        sl = slice(c * NS, (c + 1) * NS)
        m = dict(static)
        m["m0"] = build_m0(x[sl], le[sl])
        in_maps.append(m)
    return in_maps


def kernel(x, t, labels, label_emb, ew1, eb1, ew2, eb2, bw1, bb1,
           qw, qb, kw, kb, vw, vb, gamma, bw2, bb2, dtw, dtb, dcw, dcb):
    del t
    x = np.asarray(x, np.float32)
    labels = np.asarray(labels)
    label_emb = np.asarray(label_emb, np.float32)
    static = prep_static(np.asarray(ew1), np.asarray(eb1), np.asarray(ew2),
                         np.asarray(eb2), np.asarray(bw1), np.asarray(bb1),
                         np.asarray(qw), np.asarray(qb), np.asarray(kw),
                         np.asarray(kb), np.asarray(vw), np.asarray(vb),
                         np.asarray(gamma), np.asarray(bw2), np.asarray(bb2),
                         np.asarray(dtw), np.asarray(dtb), np.asarray(dcw),
                         np.asarray(dcb))
    in_maps = make_in_maps(x, labels, label_emb, static)
    if "nc" not in _cache:
        _cache["nc"] = build_nc()
    nc = _cache["nc"]
    res = run_bass_kernel_spmd(nc, in_maps, core_ids=list(range(NCORES)))
    return np.concatenate([res.results[c]["out"] for c in range(NCORES)], axis=0)

